# revision 28
# baseline (speedup 1.0000x reference)
"""Trainium2 Bass kernel for CausalLocalBlock.

Reference computation (B=4, N=4096, D=256, W=7, K=15, H=1024):
    mix = causal_conv1d(x, w_mix, left_pad=2W) + b_mix
    h   = layer_norm(x + mix) * g1 + b1
    ff  = gelu(h @ w_ff1 + b_ff1) @ w_ff2 + b_ff2
    out = layer_norm(h + ff) * g2 + b2

Sharding: 8 cores, core c handles batch c//2, sequence half c%2 (2048
tokens) with a 14-token halo passed in from the host (no collectives).

On-chip layout is D-major (features on partitions, tokens on the free
dim).  Matmuls run in float32r (1 PE cycle/column; bf16 measured
SLOWER per-matmul on this part, 259 vs 232 ns at N=512).  x is the one
DMA-latency-critical input, so it ships as bf16 (chunk-contiguous, one
2104B descriptor per partition per chunk) and is widened to fp32r on
the DVE right after landing — bf16 values are exact in the fp32r grid,
so the host-side fp32r rounding only applies to the weights.

LayerNorm statistics use ones-matmuls on the PE (partition reduction +
broadcast in one op).  rstd = sqrt(D)/sqrt(T + D*eps) is computed on
the otherwise-idle GpSimd engine with the bit-trick seed + one Newton
step (max rel err ~0.17%), which keeps Ln/Exp off ScalarE entirely:
ScalarE then only ever runs Identity/Square/Gelu, all in the gelu ACT
table set, so the ~1.3us ACT_TABLE_LOAD switches disappear.  The
ops->o_sb copies run on VectorE (tensor_scalar add) for the same
reason.  Residuals and biases fold into PE taps / op fusions:
  - w_mix[14] += I                      (x + mix residual)
  - lhsT=diag(g1), rhs=hnorm            (h*g1 + ff2 accumulation)
  - g1 folded into w_ff1, c1 = b1@w_ff1 + b_ff1 folded into gelu bias
  - b1+b_ff2 added in the o-copy; g2,b2 in the final tensor_scalar

DMA priority: x chunks first on the scalar HWDGE queue, wmix tap pairs
first on the sync queue; the conv loop is tap-major so taps are needed
at ~0.9us intervals, matching their arrival.  48 ones-matmuls warm the
PE (HAM un-throttle needs ~3.4us of activity) while the first DMAs
land.  Output DMAs alternate between the two queues per D-half, and
the last chunk's ff2/LN2 stage is split into two 256-column halves so
the drain tail after the final matmul is ~3us instead of ~10us.

This walrus build encodes at most ONE sync-wait command per
instruction, so `split_multiwaits` hoists extra waits onto single-wait
NoOps after Tile scheduling.
"""

import copy
import math
import sys

if "/opt/trn_rl_repo" not in sys.path:
    sys.path.insert(0, "/opt/trn_rl_repo")

import ml_dtypes
import numpy as np

import concourse.bass as bass
import concourse.mybir as mybir
import concourse.tile as tile
from concourse.bass_utils import run_bass_kernel_spmd

B, N, D, W = 4, 4096, 256, 7
K = 2 * W + 1
H = 4 * D
EPS = 1e-5
NCORES = 8
TOK = B * N // NCORES          # 2048 tokens per core
HALO = 2 * W                   # 14
CHUNK = 512
NCHUNK = TOK // CHUNK          # 4
DH = D // 128                  # 2 partition halves of D
HJ = H // 128                  # 8 partition tiles of H
XC = CHUNK + HALO              # per-chunk x slice width
HC = CHUNK // 2                # split width for the last chunk's tail

F32 = mybir.dt.float32
F32R = mybir.dt.float32r
BF16 = mybir.dt.bfloat16
I32 = mybir.dt.int32
ACTF = mybir.ActivationFunctionType
OP = mybir.AluOpType
NP_BF16 = ml_dtypes.bfloat16
RSQRT_MAGIC = 0x5F375A86
N_WARM = 28


def round_fp32r(a):
    """Host-side RNE to the fp32r grid (low 12 mantissa bits dropped)."""
    u = np.ascontiguousarray(a, np.float32).view(np.uint32)
    r = (u.astype(np.uint64) + 0x7FF + ((u >> 12) & 1)) & 0xFFFFF000
    return r.astype(np.uint32).view(np.float32)


def split_multiwaits(nc, max_waits=1):
    """This container's walrus encodes at most one sync-wait command per
    instruction; hoist extra waits onto preceding single-wait NoOps."""
    n = 0
    new_module = copy.replace(nc.m, functions=[])
    for function in nc.m.functions:
        new_function = copy.replace(function, blocks=[])
        new_function.set_allocations_from_list(function.allocations)
        for block in function.blocks:
            new_insts = []
            for inst in block.instructions:
                si = inst.sync_info
                if si is not None and len(si.on_wait) > max_waits:
                    waits = list(si.on_wait)
                    for w in waits[:-max_waits]:
                        n += 1
                        nop = mybir.InstNoOp(name=f"WSPLIT-{n}", ins=[], outs=[])
                        nop.engine = inst.engine
                        nop.sync_info = mybir.SyncInfo(on_wait=[w], on_update=[])
                        new_insts.append(nop)
                    inst.sync_info = mybir.SyncInfo(
                        on_wait=waits[-max_waits:], on_update=list(si.on_update)
                    )
                new_insts.append(inst)
            new_function.blocks.append(copy.replace(block, instructions=new_insts))
        new_module.functions.append(new_function)
    nc.m = new_module
    return n


def build_nc():
    nc = bass.Bass()

    xP = nc.declare_dram_parameter("xP", [128, NCHUNK * DH * XC], F32, isOutput=False)
    wmix = nc.declare_dram_parameter("wmix", [128, K * DH * DH * 128], F32, isOutput=False)
    w1 = nc.declare_dram_parameter("w1", [128, DH * HJ * 128], F32, isOutput=False)
    w2 = nc.declare_dram_parameter("w2", [128, HJ * DH * 128], F32, isOutput=False)
    dg1 = nc.declare_dram_parameter("dg1", [128, DH * 128], F32, isOutput=False)
    # vecs columns: bmix(2), c1(8), g2(2), b2(2), brow(2)
    vecs = nc.declare_dram_parameter("vecs", [128, 16], F32, isOutput=False)
    outP = nc.declare_dram_parameter("outP", [128, NCHUNK * DH * CHUNK], F32, isOutput=True)

    xP_v = xP.rearrange("p (c h t) -> p c h t", c=NCHUNK, h=DH).bitcast(F32R)
    wmix_v = wmix.rearrange("p (k a b j) -> p k a b j", k=K, a=DH, b=DH).bitcast(F32R)
    outP_v = outP.rearrange("p (c h t) -> p c h t", c=NCHUNK, h=DH)

    inv_d = 1.0 / D
    # rstd = 1/sqrt(var); tv2 = 0.5*var is bit-tricked with a magic constant
    # pre-shifted by 2^22 to fold in the 1/sqrt(2): seed = M2 - (bits(tv2)>>1)
    # approximates 1/sqrt(2*tv2) = 1/sqrt(var).  One Newton step:
    # r = seed*(1.5 - tv2*seed^2).  (D*eps shifts rstd by ~2e-6 rel: dropped.)
    magic2 = RSQRT_MAGIC - 0x400000

    with tile.TileContext(nc) as tc:
        with tc.tile_pool(name="persist", bufs=1) as pers:
            # --- inputs, interleaved across the two HWDGE queues in need
            # order.  Each dma_start trigger costs ~1.1us on its issuing
            # engine, so triggers are ordered by data deadline: the conv
            # consumes wmix tap pair k at ~1.9us intervals, so pairs
            # alternate sync/scalar to double the arrival rate.
            wmix_sb = pers.tile([128, K, DH, DH, 128], F32R)
            x_sb = []
            for c in range(NCHUNK):
                xf = pers.tile([128, DH, XC], F32R, tag=f"x{c}", name=f"x{c}")
                x_sb.append(xf)
            k_edges = [0, 2, 4, 6, 8, 10, 12, 14, K]
            k_pairs = list(zip(k_edges[:-1], k_edges[1:]))

            def wpiece(i):
                k0, k1 = k_pairs[i]
                eng = nc.sync if i % 2 == 0 else nc.scalar
                eng.dma_start(out=wmix_sb[:, k0:k1], in_=wmix_v[:, k0:k1])

            # sync ring boots ~1.8us before the scalar ring; both ring
            # heads carry the conv-critical pieces (x0 halves + first taps)
            # ordered by consumption deadline
            wpiece(0)                                        # sync
            nc.scalar.dma_start(out=x_sb[0], in_=xP_v[:, 0])
            wpiece(2)                                        # sync
            wpiece(1)                                        # scalar
            wpiece(4)                                        # sync
            wpiece(3)                                        # scalar
            nc.scalar.dma_start(out=x_sb[1], in_=xP_v[:, 1])
            wpiece(6)                                        # sync
            wpiece(5)                                        # scalar
            vecs_sb = pers.tile([128, 16], F32)
            nc.sync.dma_start(out=vecs_sb, in_=vecs[:, :])
            wpiece(7)                                        # scalar
            nc.scalar.dma_start(out=x_sb[2], in_=xP_v[:, 2])
            nc.scalar.dma_start(out=x_sb[3], in_=xP_v[:, 3])
            w2_sb = pers.tile([128, HJ, DH, 128], F32R)
            nc.sync.dma_start(
                out=w2_sb,
                in_=w2.rearrange("p (j a n) -> p j a n", j=HJ, a=DH).bitcast(F32R),
            )
            w1_sb = pers.tile([128, DH, HJ, 128], F32R)
            nc.scalar.dma_start(
                out=w1_sb,
                in_=w1.rearrange("p (a j n) -> p a j n", a=DH, j=HJ).bitcast(F32R),
            )
            dg1_sb = pers.tile([128, DH, 128], F32R)
            nc.scalar.dma_start(
                out=dg1_sb, in_=dg1.rearrange("p (a n) -> p a n", a=DH).bitcast(F32R)
            )

            def wmix_tap(ki, di, do):
                return wmix_sb[:, ki, di, do, :]

            bmix_c = vecs_sb[:, 0:2]
            c1_c = vecs_sb[:, 2:10]
            g2_c = vecs_sb[:, 10:12]
            b2_c = vecs_sb[:, 12:14]
            brow_c = vecs_sb[:, 14:16]

            # constants first on the GpSimd queue (otherwise idle at boot)
            # so the PE warm-up can start as early as possible
            ones_f32 = pers.tile([128, 128], F32)
            nc.gpsimd.memset(ones_f32, 1.0)
            ones_sb = pers.tile([128, 128], F32R)
            nc.gpsimd.tensor_copy(ones_sb, ones_f32)
            c15_col = pers.tile([128, 1], F32)
            nc.gpsimd.memset(c15_col, 1.5)

            # hnorm (LN1 normalized, pre-g1/b1) and o = h + ff, whole shard
            h_sb = pers.tile([128, DH, TOK], F32R)
            o_sb = pers.tile([128, DH, TOK], F32R)

            with (
                tc.tile_pool(name="big_ps", bufs=2, space="PSUM") as big_ps,
                tc.tile_pool(name="small_ps", bufs=4, space="PSUM") as small_ps,
                tc.tile_pool(name="work", bufs=2) as work,
            ):
                # per-chunk state handed between stage emitters
                st = [dict() for _ in range(NCHUNK)]

                def warmup():
                    # dummy ones-matmuls: busy the PE from ~4us so HAM
                    # un-throttles before the first conv matmul issues
                    wps = small_ps.tile([128, CHUNK], F32, tag="small")
                    for i in range(N_WARM):
                        nc.tensor.matmul(
                            wps[:, 0:128], ones_sb, ones_sb, start=True, stop=True
                        )

                def conv_block(*cs):
                    # tap-major (optionally across a pair of chunks) so tap
                    # ki is first needed well after its wmix DMA piece lands
                    for c in cs:
                        st[c]["yps"] = big_ps.tile(
                            [128, DH, CHUNK], F32, tag="big", name=f"yps{c}"
                        )
                    for ki in range(K):
                        for di in range(DH):
                            for do in range(DH):
                                for c in cs:
                                    nc.tensor.matmul(
                                        st[c]["yps"][:, do, :],
                                        wmix_tap(ki, di, do),
                                        x_sb[c][:, di, ki : ki + CHUNK],
                                        start=(ki == 0 and di == 0),
                                        stop=(ki == K - 1 and di == DH - 1),
                                    )

                def ln_stats(c, src, src_psum, pfx, h0=0, w=CHUNK):
                    """Emit sq, stat matmuls, and T = SQ - S^2/D + D*eps for
                    `src` (subrange [h0:h0+w] of a [128, DH, CHUNK] fp32r
                    view).  If src_psum is given, also copy it into src
                    (+bmix) on ScalarE and square there; else square on
                    GpSimd (keeps VectorE free for the apply chains)."""
                    if src_psum is not None:
                        for a in range(DH):
                            nc.scalar.activation(
                                src[:, a, h0 : h0 + w], src_psum[:, a, h0 : h0 + w],
                                ACTF.Identity, bias=bmix_c[:, a : a + 1], scale=1.0,
                            )
                    sq = work.tile([128, DH, CHUNK], F32R, tag="sq")
                    for a in range(DH):
                        if src_psum is not None:
                            nc.scalar.square(
                                sq[:, a, h0 : h0 + w],
                                src[:, a, h0 : h0 + w].bitcast(F32),
                            )
                        else:
                            # split halves across GpSimd and DVE for the
                            # early chunks (latency hidden under conv/ff
                            # matmuls); GpSimd's ~1.3us fixed op overhead
                            # would gate the endgame chains, so chunks 2/3
                            # square on DVE only
                            eng = nc.gpsimd if (a == 0 and c < 2) else nc.vector
                            eng.tensor_mul(
                                sq[:, a, h0 : h0 + w],
                                src[:, a, h0 : h0 + w].bitcast(F32),
                                src[:, a, h0 : h0 + w].bitcast(F32),
                            )
                    s_ps = small_ps.tile([128, CHUNK], F32, tag="small")
                    q_ps = small_ps.tile([128, CHUNK], F32, tag="small")
                    for a in range(DH):
                        nc.tensor.matmul(
                            s_ps[:, :w], ones_sb, src[:, a, h0 : h0 + w],
                            start=(a == 0), stop=(a == DH - 1),
                        )
                    for a in range(DH):
                        nc.tensor.matmul(
                            q_ps[:, :w], ones_sb, sq[:, a, h0 : h0 + w],
                            start=(a == 0), stop=(a == DH - 1),
                        )
                    # mu = S/D;  tv2 = 0.5*var = 0.5*SQ/D - 0.5*mu^2
                    mu = work.tile([128, CHUNK], F32, tag="mu")
                    nc.vector.tensor_scalar_mul(mu[:, :w], s_ps[:, :w], inv_d)
                    t1 = work.tile([128, CHUNK], F32, tag="t1")
                    nc.vector.tensor_mul(t1[:, :w], mu[:, :w], mu[:, :w])
                    qd = work.tile([128, CHUNK], F32, tag="qd")
                    nc.vector.tensor_scalar_mul(qd[:, :w], q_ps[:, :w], 0.5 * inv_d)
                    tv = work.tile([128, CHUNK], F32, tag="tv")
                    nc.vector.scalar_tensor_tensor(
                        out=tv[:, :w], in0=t1[:, :w], scalar=-0.5,
                        in1=qd[:, :w], op0=OP.mult, op1=OP.add,
                    )
                    st[c][pfx + "mu"] = mu
                    st[c][pfx + "tv"] = tv

                def ln_rstd(c, pfx, w=CHUNK):
                    # rstd = 1/sqrt(var) = 1/sqrt(2*tv2): bit-trick seed (int
                    # TensorScalar on DVE) + one Newton step split across
                    # ScalarE (Square / Identity, both in the gelu ACT set —
                    # no table switch) and DVE.  Max rel err ~0.17%.
                    tv = st[c][pfx + "tv"]
                    sh = work.tile([128, CHUNK], F32, tag="sh")
                    nc.vector.tensor_scalar(
                        out=sh.bitcast(I32)[:, :w], in0=tv.bitcast(I32)[:, :w],
                        scalar1=1, scalar2=None, op0=OP.logical_shift_right,
                    )
                    y0 = work.tile([128, CHUNK], F32, tag="y0")
                    nc.vector.tensor_scalar(
                        out=y0.bitcast(I32)[:, :w], in0=sh.bitcast(I32)[:, :w],
                        scalar1=-1, scalar2=magic2, op0=OP.mult, op1=OP.add,
                    )
                    y2 = work.tile([128, CHUNK], F32, tag="y2")
                    nc.scalar.square(y2[:, :w], y0[:, :w])
                    e = work.tile([128, CHUNK], F32, tag="t1", name="e")
                    nc.vector.tensor_mul(e[:, :w], y2[:, :w], tv[:, :w])
                    g = work.tile([128, CHUNK], F32, tag="g")
                    nc.scalar.activation(g[:, :w], e[:, :w], ACTF.Identity,
                                         bias=c15_col[:, 0:1], scale=-1.0)
                    r = work.tile([128, CHUNK], F32, tag="r")
                    nc.vector.tensor_mul(r[:, :w], y0[:, :w], g[:, :w])
                    st[c][pfx + "r"] = r

                def ln_rstd_lnexp(c, pfx, w=CHUNK):
                    # Ln/Exp variant for phases after the last gelu (one ACT
                    # table switch total): rstd = exp(-0.5*ln(2*tv2))
                    tv = st[c][pfx + "tv"]
                    lnv = work.tile([128, CHUNK], F32, tag="sh", name="lnv")
                    nc.scalar.activation(lnv[:, :w], tv[:, :w], ACTF.Ln, scale=2.0)
                    r = work.tile([128, CHUNK], F32, tag="r")
                    nc.scalar.activation(r[:, :w], lnv[:, :w], ACTF.Exp, scale=-0.5)
                    st[c][pfx + "r"] = r

                def ln1_apply(c):
                    c0 = c * CHUNK
                    mu, r = st[c]["1mu"], st[c]["1r"]
                    ysb = st[c]["ysb"]
                    for a in range(DH):
                        t0 = work.tile([128, CHUNK], F32, tag="t0")
                        nc.vector.tensor_sub(t0, ysb[:, a, :].bitcast(F32), mu)
                        nc.vector.tensor_mul(h_sb[:, a, c0 : c0 + CHUNK], t0, r)

                def zg_block(c, h0=0, w=CHUNK):
                    c0 = c * CHUNK + h0
                    if h0 == 0:
                        st[c]["gel"] = work.tile([128, HJ, CHUNK], F32R, tag="gel", name="gel")
                    gel = st[c]["gel"]
                    for j in range(HJ):
                        zps = small_ps.tile([128, CHUNK], F32, tag="small")
                        for di in range(DH):
                            nc.tensor.matmul(
                                zps[:, :w],
                                w1_sb[:, di, j, :],
                                h_sb[:, di, c0 : c0 + w],
                                start=(di == 0), stop=(di == DH - 1),
                            )
                        nc.scalar.activation(
                            gel[:, j, h0 : h0 + w], zps[:, :w], ACTF.Gelu,
                            bias=c1_c[:, j : j + 1], scale=1.0,
                        )

                def ff2_block(c, h0=0, w=CHUNK):
                    """ff2 matmuls + o copy for columns [h0, h0+w) of chunk
                    c.  For w=CHUNK ops is [128, DH, CHUNK] (2 banks); for
                    smaller w both halves pack into one small tile."""
                    c0 = c * CHUNK + h0
                    gel = st[c]["gel"]
                    if w == CHUNK:
                        ops = big_ps.tile([128, DH, CHUNK], F32, tag="big")
                        oview = [ops[:, do, :] for do in range(DH)]
                    else:
                        ops = small_ps.tile([128, CHUNK], F32, tag="small")
                        oview = [ops[:, do * w : (do + 1) * w] for do in range(DH)]
                    for do in range(DH):
                        nc.tensor.matmul(
                            oview[do], dg1_sb[:, do, :],
                            h_sb[:, do, c0 : c0 + w],
                            start=True, stop=False,
                        )
                        for j in range(HJ):
                            nc.tensor.matmul(
                                oview[do], w2_sb[:, j, do, :],
                                gel[:, j, h0 : h0 + w],
                                start=False, stop=(j == HJ - 1),
                            )
                    for a in range(DH):
                        nc.scalar.activation(
                            o_sb[:, a, c0 : c0 + w], oview[a], ACTF.Identity,
                            bias=brow_c[:, a : a + 1], scale=1.0,
                        )

                def ln2_apply(c, h0=0, w=CHUNK, pfx="2", split=False):
                    # split=True sends the a=1 half through GpSimd so the two
                    # halves run in parallel during the drain tail
                    c0 = c * CHUNK + h0
                    mu, r = st[c][pfx + "mu"], st[c][pfx + "r"]
                    out_t = work.tile([128, DH, CHUNK], F32, tag="outsb")
                    for a in range(DH):
                        ve = nc.gpsimd if (split and a == 1) else nc.vector
                        t0 = work.tile([128, CHUNK], F32, tag="t0")
                        ve.tensor_sub(
                            t0[:, :w], o_sb[:, a, c0 : c0 + w].bitcast(F32), mu[:, :w]
                        )
                        ve.tensor_mul(t0[:, :w], t0[:, :w], r[:, :w])
                        ve.tensor_scalar(
                            out=out_t[:, a, :w], in0=t0[:, :w],
                            scalar1=g2_c[:, a : a + 1], scalar2=b2_c[:, a : a + 1],
                            op0=OP.mult, op1=OP.add,
                        )
                        # alternate output halves across the two HWDGE queues
                        eng = nc.sync if a == 0 else nc.scalar
                        eng.dma_start(
                            out=outP_v[:, c, a, h0 : h0 + w], in_=out_t[:, a, :w]
                        )

                def s1(c):
                    ysb = work.tile([128, DH, CHUNK], F32R, tag="ysb")
                    st[c]["ysb"] = ysb
                    ln_stats(c, ysb, st[c]["yps"], "1")

                def s2(c, h0=0, w=CHUNK, pfx="2"):
                    c0 = c * CHUNK
                    ln_stats(c, o_sb[:, :, c0 : c0 + CHUNK], None, pfx, h0, w)

                # --- software-pipelined emission ---
                warmup()
                conv_block(0)
                conv_block(1)
                s1(0); ln_rstd(0, "1"); ln1_apply(0)
                s1(1); ln_rstd(1, "1"); ln1_apply(1)
                conv_block(2)
                zg_block(0)
                conv_block(3)
                s1(2); ln_rstd(2, "1"); ln1_apply(2)
                s1(3); ln_rstd(3, "1"); ln1_apply(3)
                ff2_block(0)
                zg_block(1)
                ff2_block(1)
                s2(0); ln_rstd(0, "2"); ln2_apply(0)
                zg_block(2)
                ff2_block(2)
                s2(1); ln_rstd(1, "2"); ln2_apply(1)
                # Endgame: the whole chunk-3 FF pipeline runs in two column
                # halves so its serial zg->gelu->ff2->LN2 chain is shorter,
                # and chunk2/3 LN2 rstds (which fall after the last gelu) use
                # Ln/Exp with a single ACT table switch.  Apply chains split
                # their halves across DVE and GpSimd to shorten the drain.
                zg_block(3)
                s2(2)
                ff2_block(3, 0, HC)
                ff2_block(3, HC, HC)
                s2(3, 0, HC, pfx="2a")
                s2(3, HC, HC, pfx="2b")
                ln_rstd_lnexp(2, "2")
                ln2_apply(2, split=True)
                ln_rstd_lnexp(3, "2a", HC)
                ln2_apply(3, 0, HC, pfx="2a")
                ln_rstd_lnexp(3, "2b", HC)
                ln2_apply(3, HC, HC, pfx="2b")

    split_multiwaits(nc)
    return nc


def _pack_inputs(x, w_mix, b_mix, g1, b1, w_ff1, b_ff1, w_ff2, b_ff2, g2, b2):
    """Host-side packing shared by all cores (weights) + per-core shards."""
    f32 = np.float32
    f64 = np.float64
    Wm = np.array(w_mix, dtype=f64).copy()
    Wm[K - 1] += np.eye(D)
    wmix_p = round_fp32r(
        Wm.reshape(K, DH, 128, DH, 128).transpose(2, 0, 1, 3, 4).reshape(128, -1)
    )
    W1g = np.array(g1, f64)[:, None] * np.array(w_ff1, f64)
    w1_p = round_fp32r(
        W1g.reshape(DH, 128, HJ, 128).transpose(1, 0, 2, 3).reshape(128, -1)
    )
    w2_p = round_fp32r(
        np.array(w_ff2, f64).reshape(HJ, 128, DH, 128).transpose(1, 0, 2, 3).reshape(128, -1)
    )
    dg1_p = np.zeros((128, DH, 128), f32)
    for a in range(DH):
        dg1_p[np.arange(128), a, np.arange(128)] = np.array(g1, f32)[a * 128 : (a + 1) * 128]
    dg1_p = round_fp32r(dg1_p.reshape(128, -1))
    c1 = (np.array(b1, f64) @ np.array(w_ff1, f64) + np.array(b_ff1, f64)).astype(f32)
    vecs_p = np.zeros((128, 16), f32)
    vecs_p[:, 0:2] = np.array(b_mix, f32).reshape(DH, 128).T
    vecs_p[:, 2:10] = c1.reshape(HJ, 128).T
    vecs_p[:, 10:12] = np.array(g2, f32).reshape(DH, 128).T
    vecs_p[:, 12:14] = np.array(b2, f32).reshape(DH, 128).T
    vecs_p[:, 14:16] = (
        (np.array(b1, f64) + np.array(b_ff2, f64)).astype(f32).reshape(DH, 128).T
    )

    shared = {
        "wmix": wmix_p, "w1": w1_p, "w2": w2_p, "dg1": dg1_p,
        "vecs": vecs_p,
    }
    in_maps = []
    x = np.array(x, f32)
    for core in range(NCORES):
        b, half = divmod(core, 2)
        start = half * TOK
        xT_shard = np.zeros((D, HALO + TOK), f32)
        xT_shard[:, HALO:] = x[b, start : start + TOK].T
        if start > 0:
            xT_shard[:, :HALO] = x[b, start - HALO : start].T
        # chunk-contiguous windows: partition p row = (c, h, t) with
        # xP[p, c, h, t] = x_shard_T[h*128+p, c*CHUNK + t],  t in [0, XC)
        xw = np.stack(
            [xT_shard[:, c * CHUNK : c * CHUNK + XC] for c in range(NCHUNK)], axis=1
        )  # [D, NCHUNK, XC]
        xPa = round_fp32r(
            xw.reshape(DH, 128, NCHUNK, XC).transpose(1, 2, 0, 3).reshape(128, -1)
        )
        in_maps.append({"xP": xPa, **shared})
    return in_maps


_NC_CACHE = None


def _get_nc():
    global _NC_CACHE
    if _NC_CACHE is None:
        _NC_CACHE = build_nc()
    return _NC_CACHE


def run_spmd(in_maps, **kwargs):
    return run_bass_kernel_spmd(_get_nc(), in_maps, core_ids=list(range(NCORES)), **kwargs)


def assemble(results):
    out = np.empty((B, N, D), np.float32)
    for core in range(NCORES):
        b, half = divmod(core, 2)
        start = half * TOK
        o = results[core]["outP"]  # [128, NCHUNK*DH*CHUNK] f32
        oT = (
            np.asarray(o, np.float32)
            .reshape(128, NCHUNK, DH, CHUNK)
            .transpose(2, 0, 1, 3)
            .reshape(D, TOK)
        )
        out[b, start : start + TOK, :] = oT.T
    return out


def kernel(**inputs):
    res = run_spmd(_pack_inputs(**inputs))
    return assemble(res.results)


# revision 29
# speedup vs baseline: 1.0136x; 1.0136x over previous
"""Trainium2 Bass kernel for CausalLocalBlock.

Reference computation (B=4, N=4096, D=256, W=7, K=15, H=1024):
    mix = causal_conv1d(x, w_mix, left_pad=2W) + b_mix
    h   = layer_norm(x + mix) * g1 + b1
    ff  = gelu(h @ w_ff1 + b_ff1) @ w_ff2 + b_ff2
    out = layer_norm(h + ff) * g2 + b2

Sharding: 8 cores, core c handles batch c//2, sequence half c%2 (2048
tokens) with a 14-token halo passed in from the host (no collectives).

On-chip layout is D-major (features on partitions, tokens on the free
dim).  Matmuls run in float32r (1 PE cycle/column; bf16 measured
SLOWER per-matmul on this part, 259 vs 232 ns at N=512).  x is the one
DMA-latency-critical input, so it ships as bf16 (chunk-contiguous, one
2104B descriptor per partition per chunk) and is widened to fp32r on
the DVE right after landing — bf16 values are exact in the fp32r grid,
so the host-side fp32r rounding only applies to the weights.

LayerNorm statistics use ones-matmuls on the PE (partition reduction +
broadcast in one op).  rstd = sqrt(D)/sqrt(T + D*eps) is computed on
the otherwise-idle GpSimd engine with the bit-trick seed + one Newton
step (max rel err ~0.17%), which keeps Ln/Exp off ScalarE entirely:
ScalarE then only ever runs Identity/Square/Gelu, all in the gelu ACT
table set, so the ~1.3us ACT_TABLE_LOAD switches disappear.  The
ops->o_sb copies run on VectorE (tensor_scalar add) for the same
reason.  Residuals and biases fold into PE taps / op fusions:
  - w_mix[14] += I                      (x + mix residual)
  - lhsT=diag(g1), rhs=hnorm            (h*g1 + ff2 accumulation)
  - g1 folded into w_ff1, c1 = b1@w_ff1 + b_ff1 folded into gelu bias
  - b1+b_ff2 added in the o-copy; g2,b2 in the final tensor_scalar

DMA priority: x chunks first on the scalar HWDGE queue, wmix tap pairs
first on the sync queue; the conv loop is tap-major so taps are needed
at ~0.9us intervals, matching their arrival.  48 ones-matmuls warm the
PE (HAM un-throttle needs ~3.4us of activity) while the first DMAs
land.  Output DMAs alternate between the two queues per D-half, and
the last chunk's ff2/LN2 stage is split into two 256-column halves so
the drain tail after the final matmul is ~3us instead of ~10us.

This walrus build encodes at most ONE sync-wait command per
instruction, so `split_multiwaits` hoists extra waits onto single-wait
NoOps after Tile scheduling.
"""

import copy
import math
import sys

if "/opt/trn_rl_repo" not in sys.path:
    sys.path.insert(0, "/opt/trn_rl_repo")

import ml_dtypes
import numpy as np

import concourse.bass as bass
import concourse.mybir as mybir
import concourse.tile as tile
from concourse.bass_utils import run_bass_kernel_spmd

B, N, D, W = 4, 4096, 256, 7
K = 2 * W + 1
H = 4 * D
EPS = 1e-5
NCORES = 8
TOK = B * N // NCORES          # 2048 tokens per core
HALO = 2 * W                   # 14
CHUNK = 512
NCHUNK = TOK // CHUNK          # 4
DH = D // 128                  # 2 partition halves of D
HJ = H // 128                  # 8 partition tiles of H
XC = CHUNK + HALO              # per-chunk x slice width
HC = CHUNK // 2                # split width for the last chunk's tail

F32 = mybir.dt.float32
F32R = mybir.dt.float32r
BF16 = mybir.dt.bfloat16
I32 = mybir.dt.int32
ACTF = mybir.ActivationFunctionType
OP = mybir.AluOpType
NP_BF16 = ml_dtypes.bfloat16
RSQRT_MAGIC = 0x5F375A86
N_WARM = 12


def round_fp32r(a):
    """Host-side RNE to the fp32r grid (low 12 mantissa bits dropped)."""
    u = np.ascontiguousarray(a, np.float32).view(np.uint32)
    r = (u.astype(np.uint64) + 0x7FF + ((u >> 12) & 1)) & 0xFFFFF000
    return r.astype(np.uint32).view(np.float32)


def split_multiwaits(nc, max_waits=1):
    """This container's walrus encodes at most one sync-wait command per
    instruction; hoist extra waits onto preceding single-wait NoOps."""
    n = 0
    new_module = copy.replace(nc.m, functions=[])
    for function in nc.m.functions:
        new_function = copy.replace(function, blocks=[])
        new_function.set_allocations_from_list(function.allocations)
        for block in function.blocks:
            new_insts = []
            for inst in block.instructions:
                si = inst.sync_info
                if si is not None and len(si.on_wait) > max_waits:
                    waits = list(si.on_wait)
                    for w in waits[:-max_waits]:
                        n += 1
                        nop = mybir.InstNoOp(name=f"WSPLIT-{n}", ins=[], outs=[])
                        nop.engine = inst.engine
                        nop.sync_info = mybir.SyncInfo(on_wait=[w], on_update=[])
                        new_insts.append(nop)
                    inst.sync_info = mybir.SyncInfo(
                        on_wait=waits[-max_waits:], on_update=list(si.on_update)
                    )
                new_insts.append(inst)
            new_function.blocks.append(copy.replace(block, instructions=new_insts))
        new_module.functions.append(new_function)
    nc.m = new_module
    return n


def build_nc():
    nc = bass.Bass()

    xP = nc.declare_dram_parameter("xP", [128, NCHUNK * DH * XC], F32, isOutput=False)
    wmix = nc.declare_dram_parameter("wmix", [128, K * DH * DH * 128], F32, isOutput=False)
    w1 = nc.declare_dram_parameter("w1", [128, DH * HJ * 128], F32, isOutput=False)
    w2 = nc.declare_dram_parameter("w2", [128, HJ * DH * 128], F32, isOutput=False)
    dg1 = nc.declare_dram_parameter("dg1", [128, DH * 128], F32, isOutput=False)
    # vecs columns: bmix(2), c1(8), g2(2), b2(2), brow(2)
    vecs = nc.declare_dram_parameter("vecs", [128, 16], F32, isOutput=False)
    outP = nc.declare_dram_parameter("outP", [128, NCHUNK * DH * CHUNK], F32, isOutput=True)

    xP_v = xP.rearrange("p (c h t) -> p c h t", c=NCHUNK, h=DH).bitcast(F32R)
    wmix_v = wmix.rearrange("p (k a b j) -> p k a b j", k=K, a=DH, b=DH).bitcast(F32R)
    outP_v = outP.rearrange("p (c h t) -> p c h t", c=NCHUNK, h=DH)

    inv_d = 1.0 / D
    # rstd = 1/sqrt(var); tv2 = 0.5*var is bit-tricked with a magic constant
    # pre-shifted by 2^22 to fold in the 1/sqrt(2): seed = M2 - (bits(tv2)>>1)
    # approximates 1/sqrt(2*tv2) = 1/sqrt(var).  One Newton step:
    # r = seed*(1.5 - tv2*seed^2).  (D*eps shifts rstd by ~2e-6 rel: dropped.)
    magic2 = RSQRT_MAGIC - 0x400000

    with tile.TileContext(nc) as tc:
        with tc.tile_pool(name="persist", bufs=1) as pers:
            # --- inputs, interleaved across the two HWDGE queues in need
            # order.  Each dma_start trigger costs ~1.1us on its issuing
            # engine, so triggers are ordered by data deadline: the conv
            # consumes wmix tap pair k at ~1.9us intervals, so pairs
            # alternate sync/scalar to double the arrival rate.
            wmix_sb = pers.tile([128, K, DH, DH, 128], F32R)
            x_sb = []
            for c in range(NCHUNK):
                xf = pers.tile([128, DH, XC], F32R, tag=f"x{c}", name=f"x{c}")
                x_sb.append(xf)
            k_edges = [0, 2, 4, 6, 8, 10, 12, 14, K]
            k_pairs = list(zip(k_edges[:-1], k_edges[1:]))

            def wpiece(i):
                k0, k1 = k_pairs[i]
                eng = nc.sync if i % 2 == 0 else nc.scalar
                eng.dma_start(out=wmix_sb[:, k0:k1], in_=wmix_v[:, k0:k1])

            # sync ring boots ~1.8us before the scalar ring; both ring
            # heads carry the conv-critical pieces (x0 halves + first taps)
            # ordered by consumption deadline
            wpiece(0)                                        # sync
            nc.scalar.dma_start(out=x_sb[0], in_=xP_v[:, 0])
            wpiece(2)                                        # sync
            wpiece(1)                                        # scalar
            wpiece(4)                                        # sync
            wpiece(3)                                        # scalar
            nc.scalar.dma_start(out=x_sb[1], in_=xP_v[:, 1])
            wpiece(6)                                        # sync
            wpiece(5)                                        # scalar
            vecs_sb = pers.tile([128, 16], F32)
            nc.sync.dma_start(out=vecs_sb, in_=vecs[:, :])
            wpiece(7)                                        # scalar
            nc.scalar.dma_start(out=x_sb[2], in_=xP_v[:, 2])
            nc.scalar.dma_start(out=x_sb[3], in_=xP_v[:, 3])
            w2_sb = pers.tile([128, HJ, DH, 128], F32R)
            nc.sync.dma_start(
                out=w2_sb,
                in_=w2.rearrange("p (j a n) -> p j a n", j=HJ, a=DH).bitcast(F32R),
            )
            w1_sb = pers.tile([128, DH, HJ, 128], F32R)
            nc.scalar.dma_start(
                out=w1_sb,
                in_=w1.rearrange("p (a j n) -> p a j n", a=DH, j=HJ).bitcast(F32R),
            )
            dg1_sb = pers.tile([128, DH, 128], F32R)
            nc.scalar.dma_start(
                out=dg1_sb, in_=dg1.rearrange("p (a n) -> p a n", a=DH).bitcast(F32R)
            )

            def wmix_tap(ki, di, do):
                return wmix_sb[:, ki, di, do, :]

            bmix_c = vecs_sb[:, 0:2]
            c1_c = vecs_sb[:, 2:10]
            g2_c = vecs_sb[:, 10:12]
            b2_c = vecs_sb[:, 12:14]
            brow_c = vecs_sb[:, 14:16]

            # constants first on the GpSimd queue (otherwise idle at boot)
            # so the PE warm-up can start as early as possible
            ones_f32 = pers.tile([128, 128], F32)
            nc.gpsimd.memset(ones_f32, 1.0)
            ones_sb = pers.tile([128, 128], F32R)
            nc.gpsimd.tensor_copy(ones_sb, ones_f32)
            c15_col = pers.tile([128, 1], F32)
            nc.gpsimd.memset(c15_col, 1.5)

            # hnorm (LN1 normalized, pre-g1/b1) and o = h + ff, whole shard
            h_sb = pers.tile([128, DH, TOK], F32R)
            o_sb = pers.tile([128, DH, TOK], F32R)

            with (
                tc.tile_pool(name="big_ps", bufs=2, space="PSUM") as big_ps,
                tc.tile_pool(name="small_ps", bufs=4, space="PSUM") as small_ps,
                tc.tile_pool(name="work", bufs=2) as work,
            ):
                # per-chunk state handed between stage emitters
                st = [dict() for _ in range(NCHUNK)]

                def warmup():
                    # dummy ones-matmuls: busy the PE from ~4us so HAM
                    # un-throttles before the first conv matmul issues
                    wps = small_ps.tile([128, CHUNK], F32, tag="small")
                    for i in range(N_WARM):
                        nc.tensor.matmul(
                            wps[:, 0:128], ones_sb, ones_sb, start=True, stop=True
                        )

                def conv_block(*cs):
                    # tap-major (optionally across a pair of chunks) so tap
                    # ki is first needed well after its wmix DMA piece lands
                    for c in cs:
                        st[c]["yps"] = big_ps.tile(
                            [128, DH, CHUNK], F32, tag="big", name=f"yps{c}"
                        )
                    for ki in range(K):
                        for di in range(DH):
                            for do in range(DH):
                                for c in cs:
                                    nc.tensor.matmul(
                                        st[c]["yps"][:, do, :],
                                        wmix_tap(ki, di, do),
                                        x_sb[c][:, di, ki : ki + CHUNK],
                                        start=(ki == 0 and di == 0),
                                        stop=(ki == K - 1 and di == DH - 1),
                                    )

                def ln_sq(c, src, h0=0, w=CHUNK):
                    # squares only (into st[c]['sq']); lets the endgame emit
                    # both halves' squares before any stats glue so the last
                    # stats matmuls aren't gated by the DVE FIFO
                    sq = work.tile([128, DH, CHUNK], F32R, tag="sq", name="sq")
                    st[c]["sq"] = sq
                    for a in range(DH):
                        eng = nc.gpsimd if (a == 0 and c < 2) else nc.vector
                        eng.tensor_mul(
                            sq[:, a, h0 : h0 + w],
                            src[:, a, h0 : h0 + w].bitcast(F32),
                            src[:, a, h0 : h0 + w].bitcast(F32),
                        )
                    return sq

                def ln_stats(c, src, src_psum, pfx, h0=0, w=CHUNK, sq=None):
                    """Emit sq, stat matmuls, and T = SQ - S^2/D + D*eps for
                    `src` (subrange [h0:h0+w] of a [128, DH, CHUNK] fp32r
                    view).  If src_psum is given, also copy it into src
                    (+bmix) on ScalarE and square there; else square on
                    GpSimd (keeps VectorE free for the apply chains)."""
                    if src_psum is not None:
                        for a in range(DH):
                            nc.scalar.activation(
                                src[:, a, h0 : h0 + w], src_psum[:, a, h0 : h0 + w],
                                ACTF.Identity, bias=bmix_c[:, a : a + 1], scale=1.0,
                            )
                    if sq is None:
                        sq = work.tile([128, DH, CHUNK], F32R, tag="sq", name="sq")
                        for a in range(DH):
                            if src_psum is not None:
                                nc.scalar.square(
                                    sq[:, a, h0 : h0 + w],
                                    src[:, a, h0 : h0 + w].bitcast(F32),
                                )
                            else:
                                eng = nc.gpsimd if (a == 0 and c < 2) else nc.vector
                                eng.tensor_mul(
                                    sq[:, a, h0 : h0 + w],
                                    src[:, a, h0 : h0 + w].bitcast(F32),
                                    src[:, a, h0 : h0 + w].bitcast(F32),
                                )
                    s_ps = small_ps.tile([128, CHUNK], F32, tag="small")
                    q_ps = small_ps.tile([128, CHUNK], F32, tag="small")
                    for a in range(DH):
                        nc.tensor.matmul(
                            s_ps[:, :w], ones_sb, src[:, a, h0 : h0 + w],
                            start=(a == 0), stop=(a == DH - 1),
                        )
                    for a in range(DH):
                        nc.tensor.matmul(
                            q_ps[:, :w], ones_sb, sq[:, a, h0 : h0 + w],
                            start=(a == 0), stop=(a == DH - 1),
                        )
                    # mu = S/D;  tv2 = 0.5*var = 0.5*SQ/D - 0.5*mu^2
                    mu = work.tile([128, CHUNK], F32, tag="mu")
                    nc.vector.tensor_scalar_mul(mu[:, :w], s_ps[:, :w], inv_d)
                    t1 = work.tile([128, CHUNK], F32, tag="t1")
                    nc.vector.tensor_mul(t1[:, :w], mu[:, :w], mu[:, :w])
                    qd = work.tile([128, CHUNK], F32, tag="qd")
                    nc.vector.tensor_scalar_mul(qd[:, :w], q_ps[:, :w], 0.5 * inv_d)
                    tv = work.tile([128, CHUNK], F32, tag="tv")
                    nc.vector.scalar_tensor_tensor(
                        out=tv[:, :w], in0=t1[:, :w], scalar=-0.5,
                        in1=qd[:, :w], op0=OP.mult, op1=OP.add,
                    )
                    st[c][pfx + "mu"] = mu
                    st[c][pfx + "tv"] = tv

                def ln_rstd(c, pfx, w=CHUNK):
                    # rstd = 1/sqrt(var) = 1/sqrt(2*tv2): bit-trick seed (int
                    # TensorScalar on DVE) + one Newton step split across
                    # ScalarE (Square / Identity, both in the gelu ACT set —
                    # no table switch) and DVE.  Max rel err ~0.17%.
                    tv = st[c][pfx + "tv"]
                    sh = work.tile([128, CHUNK], F32, tag="sh")
                    nc.vector.tensor_scalar(
                        out=sh.bitcast(I32)[:, :w], in0=tv.bitcast(I32)[:, :w],
                        scalar1=1, scalar2=None, op0=OP.logical_shift_right,
                    )
                    y0 = work.tile([128, CHUNK], F32, tag="y0")
                    nc.vector.tensor_scalar(
                        out=y0.bitcast(I32)[:, :w], in0=sh.bitcast(I32)[:, :w],
                        scalar1=-1, scalar2=magic2, op0=OP.mult, op1=OP.add,
                    )
                    y2 = work.tile([128, CHUNK], F32, tag="y2")
                    nc.scalar.square(y2[:, :w], y0[:, :w])
                    e = work.tile([128, CHUNK], F32, tag="t1", name="e")
                    nc.vector.tensor_mul(e[:, :w], y2[:, :w], tv[:, :w])
                    g = work.tile([128, CHUNK], F32, tag="g")
                    nc.scalar.activation(g[:, :w], e[:, :w], ACTF.Identity,
                                         bias=c15_col[:, 0:1], scale=-1.0)
                    r = work.tile([128, CHUNK], F32, tag="r")
                    nc.vector.tensor_mul(r[:, :w], y0[:, :w], g[:, :w])
                    st[c][pfx + "r"] = r

                def ln_rstd_lnexp(c, pfx, w=CHUNK):
                    # Ln/Exp variant for phases after the last gelu (one ACT
                    # table switch total): rstd = exp(-0.5*ln(2*tv2))
                    tv = st[c][pfx + "tv"]
                    lnv = work.tile([128, CHUNK], F32, tag="sh", name="lnv")
                    nc.scalar.activation(lnv[:, :w], tv[:, :w], ACTF.Ln, scale=2.0)
                    r = work.tile([128, CHUNK], F32, tag="r")
                    nc.scalar.activation(r[:, :w], lnv[:, :w], ACTF.Exp, scale=-0.5)
                    st[c][pfx + "r"] = r

                def ln1_apply(c):
                    c0 = c * CHUNK
                    mu, r = st[c]["1mu"], st[c]["1r"]
                    ysb = st[c]["ysb"]
                    for a in range(DH):
                        t0 = work.tile([128, CHUNK], F32, tag="t0")
                        nc.vector.tensor_sub(t0, ysb[:, a, :].bitcast(F32), mu)
                        nc.vector.tensor_mul(h_sb[:, a, c0 : c0 + CHUNK], t0, r)

                def zg_block(c, h0=0, w=CHUNK):
                    c0 = c * CHUNK + h0
                    if h0 == 0:
                        st[c]["gel"] = work.tile([128, HJ, CHUNK], F32R, tag="gel", name="gel")
                    gel = st[c]["gel"]
                    for j in range(HJ):
                        zps = small_ps.tile([128, CHUNK], F32, tag="small")
                        for di in range(DH):
                            nc.tensor.matmul(
                                zps[:, :w],
                                w1_sb[:, di, j, :],
                                h_sb[:, di, c0 : c0 + w],
                                start=(di == 0), stop=(di == DH - 1),
                            )
                        nc.scalar.activation(
                            gel[:, j, h0 : h0 + w], zps[:, :w], ACTF.Gelu,
                            bias=c1_c[:, j : j + 1], scale=1.0,
                        )

                def ff2_block(c, h0=0, w=CHUNK):
                    """ff2 matmuls + o copy for columns [h0, h0+w) of chunk
                    c.  For w=CHUNK ops is [128, DH, CHUNK] (2 banks); for
                    smaller w both halves pack into one small tile."""
                    c0 = c * CHUNK + h0
                    gel = st[c]["gel"]
                    if w == CHUNK:
                        ops = big_ps.tile([128, DH, CHUNK], F32, tag="big")
                        oview = [ops[:, do, :] for do in range(DH)]
                    else:
                        ops = small_ps.tile([128, CHUNK], F32, tag="small")
                        oview = [ops[:, do * w : (do + 1) * w] for do in range(DH)]
                    for do in range(DH):
                        nc.tensor.matmul(
                            oview[do], dg1_sb[:, do, :],
                            h_sb[:, do, c0 : c0 + w],
                            start=True, stop=False,
                        )
                        for j in range(HJ):
                            nc.tensor.matmul(
                                oview[do], w2_sb[:, j, do, :],
                                gel[:, j, h0 : h0 + w],
                                start=False, stop=(j == HJ - 1),
                            )
                    for a in range(DH):
                        nc.scalar.activation(
                            o_sb[:, a, c0 : c0 + w], oview[a], ACTF.Identity,
                            bias=brow_c[:, a : a + 1], scale=1.0,
                        )

                def ln2_apply(c, h0=0, w=CHUNK, pfx="2", split=False):
                    # split=True sends the a=1 half through GpSimd so the two
                    # halves run in parallel during the drain tail
                    c0 = c * CHUNK + h0
                    mu, r = st[c][pfx + "mu"], st[c][pfx + "r"]
                    out_t = work.tile([128, DH, CHUNK], F32, tag="outsb")
                    for a in range(DH):
                        ve = nc.gpsimd if (split and a == 1) else nc.vector
                        t0 = work.tile([128, CHUNK], F32, tag="t0")
                        ve.tensor_sub(
                            t0[:, :w], o_sb[:, a, c0 : c0 + w].bitcast(F32), mu[:, :w]
                        )
                        ve.tensor_mul(t0[:, :w], t0[:, :w], r[:, :w])
                        ve.tensor_scalar(
                            out=out_t[:, a, :w], in0=t0[:, :w],
                            scalar1=g2_c[:, a : a + 1], scalar2=b2_c[:, a : a + 1],
                            op0=OP.mult, op1=OP.add,
                        )
                        # alternate output halves across the two HWDGE queues
                        eng = nc.sync if a == 0 else nc.scalar
                        eng.dma_start(
                            out=outP_v[:, c, a, h0 : h0 + w], in_=out_t[:, a, :w]
                        )

                def s1(c):
                    ysb = work.tile([128, DH, CHUNK], F32R, tag="ysb")
                    st[c]["ysb"] = ysb
                    ln_stats(c, ysb, st[c]["yps"], "1")

                def s2(c, h0=0, w=CHUNK, pfx="2", sq=None):
                    c0 = c * CHUNK
                    ln_stats(c, o_sb[:, :, c0 : c0 + CHUNK], None, pfx, h0, w, sq=sq)

                # --- software-pipelined emission ---
                warmup()
                conv_block(0)
                conv_block(1)
                s1(0); ln_rstd(0, "1"); ln1_apply(0)
                s1(1); ln_rstd(1, "1"); ln1_apply(1)
                conv_block(2)
                zg_block(0)
                conv_block(3)
                s1(2); ln_rstd(2, "1"); ln1_apply(2)
                s1(3); ln_rstd(3, "1"); ln1_apply(3)
                ff2_block(0)
                zg_block(1)
                ff2_block(1)
                s2(0); ln_rstd(0, "2"); ln2_apply(0)
                zg_block(2)
                ff2_block(2)
                s2(1); ln_rstd(1, "2"); ln2_apply(1)
                # Endgame: the whole chunk-3 FF pipeline runs in two column
                # halves so its serial zg->gelu->ff2->LN2 chain is shorter,
                # and chunk2/3 LN2 rstds (which fall after the last gelu) use
                # Ln/Exp with a single ACT table switch.  Apply chains split
                # their halves across DVE and GpSimd to shorten the drain.
                zg_block(3)
                s2(2)
                ff2_block(3, 0, HC)
                ff2_block(3, HC, HC)
                o3 = o_sb[:, :, 3 * CHUNK : 4 * CHUNK]
                sq3a = ln_sq(3, o3, 0, HC)
                sq3b = ln_sq(3, o3, HC, HC)
                s2(3, 0, HC, pfx="2a", sq=sq3a)
                s2(3, HC, HC, pfx="2b", sq=sq3b)
                ln_rstd_lnexp(2, "2")
                ln2_apply(2, split=True)
                ln_rstd_lnexp(3, "2a", HC)
                ln2_apply(3, 0, HC, pfx="2a")
                ln_rstd_lnexp(3, "2b", HC)
                ln2_apply(3, HC, HC, pfx="2b")

    split_multiwaits(nc)
    return nc


def _pack_inputs(x, w_mix, b_mix, g1, b1, w_ff1, b_ff1, w_ff2, b_ff2, g2, b2):
    """Host-side packing shared by all cores (weights) + per-core shards."""
    f32 = np.float32
    f64 = np.float64
    Wm = np.array(w_mix, dtype=f64).copy()
    Wm[K - 1] += np.eye(D)
    wmix_p = round_fp32r(
        Wm.reshape(K, DH, 128, DH, 128).transpose(2, 0, 1, 3, 4).reshape(128, -1)
    )
    W1g = np.array(g1, f64)[:, None] * np.array(w_ff1, f64)
    w1_p = round_fp32r(
        W1g.reshape(DH, 128, HJ, 128).transpose(1, 0, 2, 3).reshape(128, -1)
    )
    w2_p = round_fp32r(
        np.array(w_ff2, f64).reshape(HJ, 128, DH, 128).transpose(1, 0, 2, 3).reshape(128, -1)
    )
    dg1_p = np.zeros((128, DH, 128), f32)
    for a in range(DH):
        dg1_p[np.arange(128), a, np.arange(128)] = np.array(g1, f32)[a * 128 : (a + 1) * 128]
    dg1_p = round_fp32r(dg1_p.reshape(128, -1))
    c1 = (np.array(b1, f64) @ np.array(w_ff1, f64) + np.array(b_ff1, f64)).astype(f32)
    vecs_p = np.zeros((128, 16), f32)
    vecs_p[:, 0:2] = np.array(b_mix, f32).reshape(DH, 128).T
    vecs_p[:, 2:10] = c1.reshape(HJ, 128).T
    vecs_p[:, 10:12] = np.array(g2, f32).reshape(DH, 128).T
    vecs_p[:, 12:14] = np.array(b2, f32).reshape(DH, 128).T
    vecs_p[:, 14:16] = (
        (np.array(b1, f64) + np.array(b_ff2, f64)).astype(f32).reshape(DH, 128).T
    )

    shared = {
        "wmix": wmix_p, "w1": w1_p, "w2": w2_p, "dg1": dg1_p,
        "vecs": vecs_p,
    }
    in_maps = []
    x = np.array(x, f32)
    for core in range(NCORES):
        b, half = divmod(core, 2)
        start = half * TOK
        xT_shard = np.zeros((D, HALO + TOK), f32)
        xT_shard[:, HALO:] = x[b, start : start + TOK].T
        if start > 0:
            xT_shard[:, :HALO] = x[b, start - HALO : start].T
        # chunk-contiguous windows: partition p row = (c, h, t) with
        # xP[p, c, h, t] = x_shard_T[h*128+p, c*CHUNK + t],  t in [0, XC)
        xw = np.stack(
            [xT_shard[:, c * CHUNK : c * CHUNK + XC] for c in range(NCHUNK)], axis=1
        )  # [D, NCHUNK, XC]
        xPa = round_fp32r(
            xw.reshape(DH, 128, NCHUNK, XC).transpose(1, 2, 0, 3).reshape(128, -1)
        )
        in_maps.append({"xP": xPa, **shared})
    return in_maps


_NC_CACHE = None


def _get_nc():
    global _NC_CACHE
    if _NC_CACHE is None:
        _NC_CACHE = build_nc()
    return _NC_CACHE


def run_spmd(in_maps, **kwargs):
    return run_bass_kernel_spmd(_get_nc(), in_maps, core_ids=list(range(NCORES)), **kwargs)


def assemble(results):
    out = np.empty((B, N, D), np.float32)
    for core in range(NCORES):
        b, half = divmod(core, 2)
        start = half * TOK
        o = results[core]["outP"]  # [128, NCHUNK*DH*CHUNK] f32
        oT = (
            np.asarray(o, np.float32)
            .reshape(128, NCHUNK, DH, CHUNK)
            .transpose(2, 0, 1, 3)
            .reshape(D, TOK)
        )
        out[b, start : start + TOK, :] = oT.T
    return out


def kernel(**inputs):
    res = run_spmd(_pack_inputs(**inputs))
    return assemble(res.results)


# revision 30
# speedup vs baseline: 1.0150x; 1.0013x over previous
"""Trainium2 Bass kernel for CausalLocalBlock.

Reference computation (B=4, N=4096, D=256, W=7, K=15, H=1024):
    mix = causal_conv1d(x, w_mix, left_pad=2W) + b_mix
    h   = layer_norm(x + mix) * g1 + b1
    ff  = gelu(h @ w_ff1 + b_ff1) @ w_ff2 + b_ff2
    out = layer_norm(h + ff) * g2 + b2

Sharding: 8 cores, core c handles batch c//2, sequence half c%2 (2048
tokens) with a 14-token halo passed in from the host (no collectives).

On-chip layout is D-major (features on partitions, tokens on the free
dim).  Matmuls run in float32r (1 PE cycle/column; bf16 measured
SLOWER per-matmul on this part, 259 vs 232 ns at N=512); all inputs are
fp32r-rounded on the host and land directly in fp32r tiles.

LayerNorm statistics use ones-matmuls on the PE (partition reduction +
broadcast in one op).  rstd = 1/sqrt(var) is computed WITHOUT Ln/Exp
for every phase that runs while gelus are still being issued: a
bit-trick seed (two int TensorScalar ops on DVE, magic pre-shifted to
fold in the 1/sqrt(2) of tv2 = 0.5*var) plus one Newton step whose
Square/Identity legs run on ScalarE — every ScalarE op stays in the
gelu ACT table set, so no ACT_TABLE_LOAD switches occur mid-kernel.
The LN2 phases of chunks 2-3 fall after the last gelu and use plain
Ln/Exp with a single table switch hidden under the last stats matmuls.
Residuals and biases fold into PE taps / op fusions:
  - w_mix[14] += I                      (x + mix residual)
  - lhsT=diag(g1), rhs=hnorm            (h*g1 + ff2 accumulation)
  - g1 folded into w_ff1, c1 = b1@w_ff1 + b_ff1 folded into gelu bias
  - b1+b_ff2 added in the o-copy; g2,b2 in the final tensor_scalar

DMA trigger instructions cost ~1us each on their issuing engine and
the two HWDGE rings boot ~2us apart, so triggers are laid out by data
deadline: first wmix tap pair + x0 at the ring heads, remaining tap
pairs alternating rings to double their arrival rate against the
tap-major conv's consumption.  A short burst of ones-matmuls warms the
PE (HAM un-throttle needs ~3.4us of busy) while the first DMAs land.
x ships chunk-contiguous (one 4208B descriptor per partition per
chunk); outputs are packed the same way and alternate rings per
D-half.  The last chunk's ff2/LN2 stage runs in two 256-column halves
(both squares emitted before any stats glue) so the drain tail after
the final matmul is short; chunk 2's apply splits its halves across
DVE and GpSimd.  GpSimd tensor ops carry ~1.3us fixed overhead, so it
only takes work whose latency hides under matmul floods.

This walrus build encodes at most ONE sync-wait command per
instruction, so `split_multiwaits` hoists extra waits onto single-wait
NoOps after Tile scheduling.
"""

import copy
import math
import sys

if "/opt/trn_rl_repo" not in sys.path:
    sys.path.insert(0, "/opt/trn_rl_repo")

import ml_dtypes
import numpy as np

import concourse.bass as bass
import concourse.mybir as mybir
import concourse.tile as tile
from concourse.bass_utils import run_bass_kernel_spmd

B, N, D, W = 4, 4096, 256, 7
K = 2 * W + 1
H = 4 * D
EPS = 1e-5
NCORES = 8
TOK = B * N // NCORES          # 2048 tokens per core
HALO = 2 * W                   # 14
CHUNK = 512
NCHUNK = TOK // CHUNK          # 4
DH = D // 128                  # 2 partition halves of D
HJ = H // 128                  # 8 partition tiles of H
XC = CHUNK + HALO              # per-chunk x slice width
HC = CHUNK // 2                # split width for the last chunk's tail

F32 = mybir.dt.float32
F32R = mybir.dt.float32r
BF16 = mybir.dt.bfloat16
I32 = mybir.dt.int32
ACTF = mybir.ActivationFunctionType
OP = mybir.AluOpType
NP_BF16 = ml_dtypes.bfloat16
RSQRT_MAGIC = 0x5F375A86
N_WARM = 12


def round_fp32r(a):
    """Host-side RNE to the fp32r grid (low 12 mantissa bits dropped)."""
    u = np.ascontiguousarray(a, np.float32).view(np.uint32)
    r = (u.astype(np.uint64) + 0x7FF + ((u >> 12) & 1)) & 0xFFFFF000
    return r.astype(np.uint32).view(np.float32)


def split_multiwaits(nc, max_waits=1):
    """This container's walrus encodes at most one sync-wait command per
    instruction; hoist extra waits onto preceding single-wait NoOps."""
    n = 0
    new_module = copy.replace(nc.m, functions=[])
    for function in nc.m.functions:
        new_function = copy.replace(function, blocks=[])
        new_function.set_allocations_from_list(function.allocations)
        for block in function.blocks:
            new_insts = []
            for inst in block.instructions:
                si = inst.sync_info
                if si is not None and len(si.on_wait) > max_waits:
                    waits = list(si.on_wait)
                    for w in waits[:-max_waits]:
                        n += 1
                        nop = mybir.InstNoOp(name=f"WSPLIT-{n}", ins=[], outs=[])
                        nop.engine = inst.engine
                        nop.sync_info = mybir.SyncInfo(on_wait=[w], on_update=[])
                        new_insts.append(nop)
                    inst.sync_info = mybir.SyncInfo(
                        on_wait=waits[-max_waits:], on_update=list(si.on_update)
                    )
                new_insts.append(inst)
            new_function.blocks.append(copy.replace(block, instructions=new_insts))
        new_module.functions.append(new_function)
    nc.m = new_module
    return n


def build_nc():
    nc = bass.Bass()

    xP = nc.declare_dram_parameter("xP", [128, NCHUNK * DH * XC], F32, isOutput=False)
    wmix = nc.declare_dram_parameter("wmix", [128, K * DH * DH * 128], F32, isOutput=False)
    w1 = nc.declare_dram_parameter("w1", [128, DH * HJ * 128], F32, isOutput=False)
    w2 = nc.declare_dram_parameter("w2", [128, HJ * DH * 128], F32, isOutput=False)
    dg1 = nc.declare_dram_parameter("dg1", [128, DH * 128], F32, isOutput=False)
    # vecs columns: bmix(2), c1(8), g2(2), b2(2), brow(2)
    vecs = nc.declare_dram_parameter("vecs", [128, 16], F32, isOutput=False)
    outP = nc.declare_dram_parameter("outP", [128, NCHUNK * DH * CHUNK], F32, isOutput=True)

    xP_v = xP.rearrange("p (c h t) -> p c h t", c=NCHUNK, h=DH).bitcast(F32R)
    wmix_v = wmix.rearrange("p (k a b j) -> p k a b j", k=K, a=DH, b=DH).bitcast(F32R)
    outP_v = outP.rearrange("p (c h t) -> p c h t", c=NCHUNK, h=DH)

    inv_d = 1.0 / D
    # rstd = 1/sqrt(var); tv2 = 0.5*var is bit-tricked with a magic constant
    # pre-shifted by 2^22 to fold in the 1/sqrt(2): seed = M2 - (bits(tv2)>>1)
    # approximates 1/sqrt(2*tv2) = 1/sqrt(var).  One Newton step:
    # r = seed*(1.5 - tv2*seed^2).  (D*eps shifts rstd by ~2e-6 rel: dropped.)
    magic2 = RSQRT_MAGIC - 0x400000

    with tile.TileContext(nc) as tc:
        with tc.tile_pool(name="persist", bufs=1) as pers:
            # --- inputs, interleaved across the two HWDGE queues in need
            # order.  Each dma_start trigger costs ~1.1us on its issuing
            # engine, so triggers are ordered by data deadline: the conv
            # consumes wmix tap pair k at ~1.9us intervals, so pairs
            # alternate sync/scalar to double the arrival rate.
            wmix_sb = pers.tile([128, K, DH, DH, 128], F32R)
            x_sb = []
            for c in range(NCHUNK):
                xf = pers.tile([128, DH, XC], F32R, tag=f"x{c}", name=f"x{c}")
                x_sb.append(xf)
            k_edges = [0, 2, 4, 6, 8, 10, 12, 14, K]
            k_pairs = list(zip(k_edges[:-1], k_edges[1:]))

            def wpiece(i):
                k0, k1 = k_pairs[i]
                eng = nc.sync if i % 2 == 0 else nc.scalar
                eng.dma_start(out=wmix_sb[:, k0:k1], in_=wmix_v[:, k0:k1])

            # sync ring boots ~1.8us before the scalar ring; both ring
            # heads carry the conv-critical pieces (x0 halves + first taps)
            # ordered by consumption deadline
            wpiece(0)                                        # sync
            nc.scalar.dma_start(out=x_sb[0], in_=xP_v[:, 0])
            wpiece(2)                                        # sync
            wpiece(1)                                        # scalar
            wpiece(4)                                        # sync
            wpiece(3)                                        # scalar
            nc.scalar.dma_start(out=x_sb[1], in_=xP_v[:, 1])
            wpiece(6)                                        # sync
            wpiece(5)                                        # scalar
            vecs_sb = pers.tile([128, 16], F32)
            nc.sync.dma_start(out=vecs_sb, in_=vecs[:, :])
            wpiece(7)                                        # scalar
            nc.scalar.dma_start(out=x_sb[2], in_=xP_v[:, 2])
            nc.scalar.dma_start(out=x_sb[3], in_=xP_v[:, 3])
            w2_sb = pers.tile([128, HJ, DH, 128], F32R)
            nc.sync.dma_start(
                out=w2_sb,
                in_=w2.rearrange("p (j a n) -> p j a n", j=HJ, a=DH).bitcast(F32R),
            )
            w1_sb = pers.tile([128, DH, HJ, 128], F32R)
            nc.scalar.dma_start(
                out=w1_sb,
                in_=w1.rearrange("p (a j n) -> p a j n", a=DH, j=HJ).bitcast(F32R),
            )
            dg1_sb = pers.tile([128, DH, 128], F32R)
            nc.scalar.dma_start(
                out=dg1_sb, in_=dg1.rearrange("p (a n) -> p a n", a=DH).bitcast(F32R)
            )

            def wmix_tap(ki, di, do):
                return wmix_sb[:, ki, di, do, :]

            bmix_c = vecs_sb[:, 0:2]
            c1_c = vecs_sb[:, 2:10]
            g2_c = vecs_sb[:, 10:12]
            b2_c = vecs_sb[:, 12:14]
            brow_c = vecs_sb[:, 14:16]

            # constants first on the GpSimd queue (otherwise idle at boot)
            # so the PE warm-up can start as early as possible
            ones_f32 = pers.tile([128, 128], F32)
            nc.gpsimd.memset(ones_f32, 1.0)
            ones_sb = pers.tile([128, 128], F32R)
            nc.gpsimd.tensor_copy(ones_sb, ones_f32)
            c15_col = pers.tile([128, 1], F32)
            nc.gpsimd.memset(c15_col, 1.5)

            # hnorm (LN1 normalized, pre-g1/b1) and o = h + ff, whole shard
            h_sb = pers.tile([128, DH, TOK], F32R)
            o_sb = pers.tile([128, DH, TOK], F32R)

            with (
                tc.tile_pool(name="big_ps", bufs=2, space="PSUM") as big_ps,
                tc.tile_pool(name="small_ps", bufs=4, space="PSUM") as small_ps,
                tc.tile_pool(name="work", bufs=2) as work,
            ):
                # per-chunk state handed between stage emitters
                st = [dict() for _ in range(NCHUNK)]

                def warmup():
                    # dummy ones-matmuls: busy the PE from ~4us so HAM
                    # un-throttles before the first conv matmul issues
                    wps = small_ps.tile([128, CHUNK], F32, tag="small")
                    for i in range(N_WARM):
                        nc.tensor.matmul(
                            wps[:, 0:128], ones_sb, ones_sb, start=True, stop=True
                        )

                def conv_block(*cs):
                    # tap-major (optionally across a pair of chunks) so tap
                    # ki is first needed well after its wmix DMA piece lands
                    for c in cs:
                        st[c]["yps"] = big_ps.tile(
                            [128, DH, CHUNK], F32, tag="big", name=f"yps{c}"
                        )
                    for ki in range(K):
                        for di in range(DH):
                            for do in range(DH):
                                for c in cs:
                                    nc.tensor.matmul(
                                        st[c]["yps"][:, do, :],
                                        wmix_tap(ki, di, do),
                                        x_sb[c][:, di, ki : ki + CHUNK],
                                        start=(ki == 0 and di == 0),
                                        stop=(ki == K - 1 and di == DH - 1),
                                    )

                def ln_sq(c, src, h0=0, w=CHUNK):
                    # squares only (into st[c]['sq']); lets the endgame emit
                    # both halves' squares before any stats glue so the last
                    # stats matmuls aren't gated by the DVE FIFO
                    sq = work.tile([128, DH, CHUNK], F32R, tag="sq", name="sq")
                    st[c]["sq"] = sq
                    for a in range(DH):
                        eng = nc.gpsimd if (a == 0 and c < 2) else nc.vector
                        eng.tensor_mul(
                            sq[:, a, h0 : h0 + w],
                            src[:, a, h0 : h0 + w].bitcast(F32),
                            src[:, a, h0 : h0 + w].bitcast(F32),
                        )
                    return sq

                def ln_stats(c, src, src_psum, pfx, h0=0, w=CHUNK, sq=None):
                    """Emit sq, stat matmuls, and T = SQ - S^2/D + D*eps for
                    `src` (subrange [h0:h0+w] of a [128, DH, CHUNK] fp32r
                    view).  If src_psum is given, also copy it into src
                    (+bmix) on ScalarE and square there; else square on
                    GpSimd (keeps VectorE free for the apply chains)."""
                    if src_psum is not None:
                        for a in range(DH):
                            nc.scalar.activation(
                                src[:, a, h0 : h0 + w], src_psum[:, a, h0 : h0 + w],
                                ACTF.Identity, bias=bmix_c[:, a : a + 1], scale=1.0,
                            )
                    if sq is None:
                        sq = work.tile([128, DH, CHUNK], F32R, tag="sq", name="sq")
                        for a in range(DH):
                            if src_psum is not None:
                                nc.scalar.square(
                                    sq[:, a, h0 : h0 + w],
                                    src[:, a, h0 : h0 + w].bitcast(F32),
                                )
                            else:
                                eng = nc.gpsimd if (a == 0 and c < 2) else nc.vector
                                eng.tensor_mul(
                                    sq[:, a, h0 : h0 + w],
                                    src[:, a, h0 : h0 + w].bitcast(F32),
                                    src[:, a, h0 : h0 + w].bitcast(F32),
                                )
                    s_ps = small_ps.tile([128, CHUNK], F32, tag="small")
                    q_ps = small_ps.tile([128, CHUNK], F32, tag="small")
                    for a in range(DH):
                        nc.tensor.matmul(
                            s_ps[:, :w], ones_sb, src[:, a, h0 : h0 + w],
                            start=(a == 0), stop=(a == DH - 1),
                        )
                    for a in range(DH):
                        nc.tensor.matmul(
                            q_ps[:, :w], ones_sb, sq[:, a, h0 : h0 + w],
                            start=(a == 0), stop=(a == DH - 1),
                        )
                    # mu = S/D;  tv2 = 0.5*var = 0.5*SQ/D - 0.5*mu^2
                    mu = work.tile([128, CHUNK], F32, tag="mu")
                    nc.vector.tensor_scalar_mul(mu[:, :w], s_ps[:, :w], inv_d)
                    t1 = work.tile([128, CHUNK], F32, tag="t1")
                    nc.vector.tensor_mul(t1[:, :w], mu[:, :w], mu[:, :w])
                    qd = work.tile([128, CHUNK], F32, tag="qd")
                    nc.vector.tensor_scalar_mul(qd[:, :w], q_ps[:, :w], 0.5 * inv_d)
                    tv = work.tile([128, CHUNK], F32, tag="tv")
                    nc.vector.scalar_tensor_tensor(
                        out=tv[:, :w], in0=t1[:, :w], scalar=-0.5,
                        in1=qd[:, :w], op0=OP.mult, op1=OP.add,
                    )
                    st[c][pfx + "mu"] = mu
                    st[c][pfx + "tv"] = tv

                def ln_rstd(c, pfx, w=CHUNK):
                    # rstd = 1/sqrt(var) = 1/sqrt(2*tv2): bit-trick seed (int
                    # TensorScalar on DVE) + one Newton step split across
                    # ScalarE (Square / Identity, both in the gelu ACT set —
                    # no table switch) and DVE.  Max rel err ~0.17%.
                    tv = st[c][pfx + "tv"]
                    sh = work.tile([128, CHUNK], F32, tag="sh")
                    nc.vector.tensor_scalar(
                        out=sh.bitcast(I32)[:, :w], in0=tv.bitcast(I32)[:, :w],
                        scalar1=1, scalar2=None, op0=OP.logical_shift_right,
                    )
                    y0 = work.tile([128, CHUNK], F32, tag="y0")
                    nc.vector.tensor_scalar(
                        out=y0.bitcast(I32)[:, :w], in0=sh.bitcast(I32)[:, :w],
                        scalar1=-1, scalar2=magic2, op0=OP.mult, op1=OP.add,
                    )
                    y2 = work.tile([128, CHUNK], F32, tag="y2")
                    nc.scalar.square(y2[:, :w], y0[:, :w])
                    e = work.tile([128, CHUNK], F32, tag="t1", name="e")
                    nc.vector.tensor_mul(e[:, :w], y2[:, :w], tv[:, :w])
                    g = work.tile([128, CHUNK], F32, tag="g")
                    nc.scalar.activation(g[:, :w], e[:, :w], ACTF.Identity,
                                         bias=c15_col[:, 0:1], scale=-1.0)
                    r = work.tile([128, CHUNK], F32, tag="r")
                    nc.vector.tensor_mul(r[:, :w], y0[:, :w], g[:, :w])
                    st[c][pfx + "r"] = r

                def ln_rstd_lnexp(c, pfx, w=CHUNK):
                    # Ln/Exp variant for phases after the last gelu (one ACT
                    # table switch total): rstd = exp(-0.5*ln(2*tv2))
                    tv = st[c][pfx + "tv"]
                    lnv = work.tile([128, CHUNK], F32, tag="sh", name="lnv")
                    nc.scalar.activation(lnv[:, :w], tv[:, :w], ACTF.Ln, scale=2.0)
                    r = work.tile([128, CHUNK], F32, tag="r")
                    nc.scalar.activation(r[:, :w], lnv[:, :w], ACTF.Exp, scale=-0.5)
                    st[c][pfx + "r"] = r

                def ln1_apply(c):
                    c0 = c * CHUNK
                    mu, r = st[c]["1mu"], st[c]["1r"]
                    ysb = st[c]["ysb"]
                    for a in range(DH):
                        t0 = work.tile([128, CHUNK], F32, tag="t0")
                        nc.vector.tensor_sub(t0, ysb[:, a, :].bitcast(F32), mu)
                        nc.vector.tensor_mul(h_sb[:, a, c0 : c0 + CHUNK], t0, r)

                def zg_block(c, h0=0, w=CHUNK):
                    c0 = c * CHUNK + h0
                    if h0 == 0:
                        st[c]["gel"] = work.tile([128, HJ, CHUNK], F32R, tag="gel", name="gel")
                    gel = st[c]["gel"]
                    for j in range(HJ):
                        zps = small_ps.tile([128, CHUNK], F32, tag="small")
                        for di in range(DH):
                            nc.tensor.matmul(
                                zps[:, :w],
                                w1_sb[:, di, j, :],
                                h_sb[:, di, c0 : c0 + w],
                                start=(di == 0), stop=(di == DH - 1),
                            )
                        nc.scalar.activation(
                            gel[:, j, h0 : h0 + w], zps[:, :w], ACTF.Gelu,
                            bias=c1_c[:, j : j + 1], scale=1.0,
                        )

                def ff2_block(c, h0=0, w=CHUNK):
                    """ff2 matmuls + o copy for columns [h0, h0+w) of chunk
                    c.  For w=CHUNK ops is [128, DH, CHUNK] (2 banks); for
                    smaller w both halves pack into one small tile."""
                    c0 = c * CHUNK + h0
                    gel = st[c]["gel"]
                    if w == CHUNK:
                        ops = big_ps.tile([128, DH, CHUNK], F32, tag="big")
                        oview = [ops[:, do, :] for do in range(DH)]
                    else:
                        ops = small_ps.tile([128, CHUNK], F32, tag="small")
                        oview = [ops[:, do * w : (do + 1) * w] for do in range(DH)]
                    for do in range(DH):
                        nc.tensor.matmul(
                            oview[do], dg1_sb[:, do, :],
                            h_sb[:, do, c0 : c0 + w],
                            start=True, stop=False,
                        )
                        for j in range(HJ):
                            nc.tensor.matmul(
                                oview[do], w2_sb[:, j, do, :],
                                gel[:, j, h0 : h0 + w],
                                start=False, stop=(j == HJ - 1),
                            )
                    for a in range(DH):
                        nc.scalar.activation(
                            o_sb[:, a, c0 : c0 + w], oview[a], ACTF.Identity,
                            bias=brow_c[:, a : a + 1], scale=1.0,
                        )

                def ln2_apply(c, h0=0, w=CHUNK, pfx="2", split=False):
                    # split=True sends the a=1 half through GpSimd so the two
                    # halves run in parallel during the drain tail
                    c0 = c * CHUNK + h0
                    mu, r = st[c][pfx + "mu"], st[c][pfx + "r"]
                    out_t = work.tile([128, DH, CHUNK], F32, tag="outsb")
                    for a in range(DH):
                        ve = nc.gpsimd if (split and a == 1) else nc.vector
                        t0 = work.tile([128, CHUNK], F32, tag="t0")
                        ve.tensor_sub(
                            t0[:, :w], o_sb[:, a, c0 : c0 + w].bitcast(F32), mu[:, :w]
                        )
                        ve.tensor_mul(t0[:, :w], t0[:, :w], r[:, :w])
                        ve.tensor_scalar(
                            out=out_t[:, a, :w], in0=t0[:, :w],
                            scalar1=g2_c[:, a : a + 1], scalar2=b2_c[:, a : a + 1],
                            op0=OP.mult, op1=OP.add,
                        )
                        # alternate output halves across the two HWDGE queues
                        eng = nc.sync if a == 0 else nc.scalar
                        eng.dma_start(
                            out=outP_v[:, c, a, h0 : h0 + w], in_=out_t[:, a, :w]
                        )

                def s1(c):
                    ysb = work.tile([128, DH, CHUNK], F32R, tag="ysb")
                    st[c]["ysb"] = ysb
                    ln_stats(c, ysb, st[c]["yps"], "1")

                def s2(c, h0=0, w=CHUNK, pfx="2", sq=None):
                    c0 = c * CHUNK
                    ln_stats(c, o_sb[:, :, c0 : c0 + CHUNK], None, pfx, h0, w, sq=sq)

                # --- software-pipelined emission ---
                warmup()
                conv_block(0)
                conv_block(1)
                s1(0); ln_rstd(0, "1"); ln1_apply(0)
                s1(1); ln_rstd(1, "1"); ln1_apply(1)
                conv_block(2)
                zg_block(0)
                conv_block(3)
                s1(2); ln_rstd(2, "1"); ln1_apply(2)
                s1(3); ln_rstd(3, "1"); ln1_apply(3)
                ff2_block(0)
                zg_block(1)
                ff2_block(1)
                s2(0); ln_rstd(0, "2"); ln2_apply(0)
                zg_block(2)
                ff2_block(2)
                s2(1); ln_rstd(1, "2"); ln2_apply(1)
                # Endgame: the whole chunk-3 FF pipeline runs in two column
                # halves so its serial zg->gelu->ff2->LN2 chain is shorter,
                # and chunk2/3 LN2 rstds (which fall after the last gelu) use
                # Ln/Exp with a single ACT table switch.  Apply chains split
                # their halves across DVE and GpSimd to shorten the drain.
                zg_block(3)
                s2(2)
                ff2_block(3, 0, HC)
                ff2_block(3, HC, HC)
                o3 = o_sb[:, :, 3 * CHUNK : 4 * CHUNK]
                sq3a = ln_sq(3, o3, 0, HC)
                sq3b = ln_sq(3, o3, HC, HC)
                s2(3, 0, HC, pfx="2a", sq=sq3a)
                s2(3, HC, HC, pfx="2b", sq=sq3b)
                ln_rstd_lnexp(2, "2")
                ln2_apply(2, split=True)
                ln_rstd_lnexp(3, "2a", HC)
                ln2_apply(3, 0, HC, pfx="2a")
                ln_rstd_lnexp(3, "2b", HC)
                ln2_apply(3, HC, HC, pfx="2b")

    split_multiwaits(nc)
    return nc


def _pack_inputs(x, w_mix, b_mix, g1, b1, w_ff1, b_ff1, w_ff2, b_ff2, g2, b2):
    """Host-side packing shared by all cores (weights) + per-core shards."""
    f32 = np.float32
    f64 = np.float64
    Wm = np.array(w_mix, dtype=f64).copy()
    Wm[K - 1] += np.eye(D)
    wmix_p = round_fp32r(
        Wm.reshape(K, DH, 128, DH, 128).transpose(2, 0, 1, 3, 4).reshape(128, -1)
    )
    W1g = np.array(g1, f64)[:, None] * np.array(w_ff1, f64)
    w1_p = round_fp32r(
        W1g.reshape(DH, 128, HJ, 128).transpose(1, 0, 2, 3).reshape(128, -1)
    )
    w2_p = round_fp32r(
        np.array(w_ff2, f64).reshape(HJ, 128, DH, 128).transpose(1, 0, 2, 3).reshape(128, -1)
    )
    dg1_p = np.zeros((128, DH, 128), f32)
    for a in range(DH):
        dg1_p[np.arange(128), a, np.arange(128)] = np.array(g1, f32)[a * 128 : (a + 1) * 128]
    dg1_p = round_fp32r(dg1_p.reshape(128, -1))
    c1 = (np.array(b1, f64) @ np.array(w_ff1, f64) + np.array(b_ff1, f64)).astype(f32)
    vecs_p = np.zeros((128, 16), f32)
    vecs_p[:, 0:2] = np.array(b_mix, f32).reshape(DH, 128).T
    vecs_p[:, 2:10] = c1.reshape(HJ, 128).T
    vecs_p[:, 10:12] = np.array(g2, f32).reshape(DH, 128).T
    vecs_p[:, 12:14] = np.array(b2, f32).reshape(DH, 128).T
    vecs_p[:, 14:16] = (
        (np.array(b1, f64) + np.array(b_ff2, f64)).astype(f32).reshape(DH, 128).T
    )

    shared = {
        "wmix": wmix_p, "w1": w1_p, "w2": w2_p, "dg1": dg1_p,
        "vecs": vecs_p,
    }
    in_maps = []
    x = np.array(x, f32)
    for core in range(NCORES):
        b, half = divmod(core, 2)
        start = half * TOK
        xT_shard = np.zeros((D, HALO + TOK), f32)
        xT_shard[:, HALO:] = x[b, start : start + TOK].T
        if start > 0:
            xT_shard[:, :HALO] = x[b, start - HALO : start].T
        # chunk-contiguous windows: partition p row = (c, h, t) with
        # xP[p, c, h, t] = x_shard_T[h*128+p, c*CHUNK + t],  t in [0, XC)
        xw = np.stack(
            [xT_shard[:, c * CHUNK : c * CHUNK + XC] for c in range(NCHUNK)], axis=1
        )  # [D, NCHUNK, XC]
        xPa = round_fp32r(
            xw.reshape(DH, 128, NCHUNK, XC).transpose(1, 2, 0, 3).reshape(128, -1)
        )
        in_maps.append({"xP": xPa, **shared})
    return in_maps


_NC_CACHE = None


def _get_nc():
    global _NC_CACHE
    if _NC_CACHE is None:
        _NC_CACHE = build_nc()
    return _NC_CACHE


def run_spmd(in_maps, **kwargs):
    return run_bass_kernel_spmd(_get_nc(), in_maps, core_ids=list(range(NCORES)), **kwargs)


def assemble(results):
    out = np.empty((B, N, D), np.float32)
    for core in range(NCORES):
        b, half = divmod(core, 2)
        start = half * TOK
        o = results[core]["outP"]  # [128, NCHUNK*DH*CHUNK] f32
        oT = (
            np.asarray(o, np.float32)
            .reshape(128, NCHUNK, DH, CHUNK)
            .transpose(2, 0, 1, 3)
            .reshape(D, TOK)
        )
        out[b, start : start + TOK, :] = oT.T
    return out


def kernel(**inputs):
    res = run_spmd(_pack_inputs(**inputs))
    return assemble(res.results)
